# revision 6
# baseline (speedup 1.0000x reference)
"""GNN message-passing kernel for Trainium2 (8 NeuronCores).

Reference computation:
    mi = segment_sum(edge_attr * x[row], col)   # [N, D]
    mo = segment_sum(edge_attr * x[col], row)   # [N, D]
    out = tanh(tanh([mi, mo, x] @ W1 + b1) @ W2 + b2)

Strategy (no collectives needed):
  - Host shards edges by destination-node range: core k owns nodes
    [k*NPC, (k+1)*NPC) and receives exactly the edges whose destination
    falls in its range (separately for the mi and mo directions).
  - Within a core, edges are further split into 4 source-chunk streams
    (dma_gather indices are int16, so the gather table is limited to
    32k rows; x is split into 4 chunks of 25600 rows).  Each stream is
    sorted by destination and packed into 128-edge tiles such that no
    destination segment crosses a tile boundary.
  - Device: dma_gather fetches x[src] rows (padded to 256B) per edge,
    DVE multiplies by edge weight, one triangular matmul per 2048
    edges produces tile-local inclusive prefix sums, which stream to
    DRAM.  Segment sums are recovered by gathering the prefix rows at
    segment-end positions and differencing consecutive values.
  - The tiny MLP runs feature-major with stationary weights, and
    results are transposed back via the tensor engine.
"""

import sys

for _p in ("/opt/trn_rl_repo",):
    if _p not in sys.path:
        sys.path.append(_p)

import numpy as np

# ---------------------------------------------------------------- geometry
N, D, O, E = 100000, 32, 64, 1600000
NCORES = 8
NPC = N // NCORES            # 12500 nodes per core
NCHUNK = 4
CH = 25600                   # x rows per source chunk
XROWS = NCHUNK * CH          # padded x table rows (102400)
DPAD = 64                    # padded x row (256B) for dma_gather
TILE = 128                   # edges per prefix tile
SUP = 2048                   # edges per supertile (one 512-col matmul)
GK = 8192                    # indices per dma_gather instruction
LSTREAM = 57344              # padded stream length (28 supertiles), per (dir, chunk)
S_SUP = LSTREAM // SUP       # 28
EPAD = 12544                 # NPC padded to 128 multiple (98 * 128)
EP_T = EPAD // 128           # 98
BG_SPLIT = 2                 # boundary gather instructions per stream
BG_K = EPAD // BG_SPLIT      # 6272 indices each
DT_SCRATCH = 1 << 15


def _host_prep_stream(dst, src, w, lo):
    """Pack one (direction, chunk) edge stream for one core.

    dst/src/w: edge arrays (dst already within [lo, lo+NPC), src already
    chunk-local in [0, CH)).  Returns gather idx [LSTREAM] int16,
    weights [LSTREAM] f32, ends_r / parity / mask per node [NPC].
    """
    order = np.argsort(dst, kind="stable")
    dst = dst[order]
    src = src[order]
    w = w[order]

    gidx = np.zeros(LSTREAM, np.int16)
    gw = np.zeros(LSTREAM, np.float32)
    end_slot = np.zeros(NPC, np.int64)      # forward-filled segment ends
    tile_start = np.zeros(NPC, bool)        # segment begins at a tile start

    # segment boundaries in the sorted edge list
    if dst.shape[0]:
        cut = np.flatnonzero(np.diff(dst)) + 1
        seg_starts = np.concatenate(([0], cut))
        seg_ends = np.concatenate((cut, [dst.shape[0]]))
    else:
        seg_starts = seg_ends = np.zeros(0, np.int64)

    pos = TILE  # tile 0 reserved as the all-zero tile
    last_end = 0
    fill_from = 0
    for s0, s1 in zip(seg_starts, seg_ends):
        node = dst[s0] - lo
        ln = s1 - s0
        assert ln <= TILE, f"segment length {ln} exceeds tile"
        if pos % TILE + ln > TILE:
            pos = (pos // TILE + 1) * TILE
        end_slot[fill_from:node] = last_end   # forward-fill empties
        tile_start[node] = pos % TILE == 0
        gidx[pos:pos + ln] = src[s0:s1].astype(np.int16)
        gw[pos:pos + ln] = w[s0:s1]
        last_end = pos + ln - 1
        end_slot[node] = last_end
        fill_from = node + 1
        pos = pos + ln
    end_slot[fill_from:] = last_end
    used = (pos // TILE + 1) * TILE
    assert used <= LSTREAM, f"stream needs {used} > LSTREAM {LSTREAM}"

    # remap stream slot -> csum DRAM row: r = (e//2048)*2048 + (e%128)*16 + (e//128)%16
    e = end_slot
    ends_r = (e // SUP) * SUP + (e % 128) * 16 + (e // 128) % 16
    parity = (ends_r % 2).astype(np.float32)
    epi = (ends_r // 2).astype(np.int16)
    assert (ends_r // 2 < 32768).all()
    # mask: subtract previous node's end unless this segment starts a tile
    present = np.zeros(NPC, bool)
    if dst.shape[0]:
        present[dst[seg_starts] - lo] = True
    mask = np.where(present & tile_start, 0.0, 1.0).astype(np.float32)
    return gidx, gw, epi, parity, mask


def _wrap16(a, p=16):
    """idx k -> [k % p (within first 16 partitions, replicated x8), k // p]."""
    k = a.shape[0]
    wtile = np.zeros((128, k // 16), a.dtype)
    for g in range(8):
        wtile[g * 16:(g + 1) * 16, :] = a.reshape(k // 16, 16).T
    return wtile


def _wrap128(a):
    """slot j -> [j % 128, j // 128]."""
    return np.ascontiguousarray(a.reshape(-1, 128).T)


def _host_prep(x, edge_index, edge_attr):
    xpad = np.zeros((XROWS, DPAD), np.float32)
    xpad[:N, :D] = x
    row = np.asarray(edge_index[0], np.int64)
    col = np.asarray(edge_index[1], np.int64)
    w = np.asarray(edge_attr[:, 0], np.float32)

    in_maps = []
    for k in range(NCORES):
        lo = k * NPC
        m = {"xpad": xpad}
        for d, (dst_all, src_all) in enumerate(((col, row), (row, col))):
            inrange = (dst_all >= lo) & (dst_all < lo + NPC)
            for c in range(NCHUNK):
                sel = inrange & (src_all // CH == c)
                gidx, gw, epi, par, msk = _host_prep_stream(
                    dst_all[sel], (src_all[sel] - c * CH), w[sel], lo)
                m[f"gi_{d}{c}"] = _wrap16(gidx)
                m[f"wv_{d}{c}"] = _wrap128(gw)
                epi_p = np.zeros(EPAD, np.int16)
                epi_p[:NPC] = epi
                par_p = np.zeros(EPAD, np.float32)
                par_p[:NPC] = par
                msk_p = np.zeros(EPAD, np.float32)
                msk_p[:NPC] = msk
                m[f"ei_{d}{c}"] = _wrap16(epi_p)
                m[f"pa_{d}{c}"] = _wrap128(par_p)
                m[f"mk_{d}{c}"] = _wrap128(msk_p)
        xr = np.zeros((EPAD, D), np.float32)
        xr[:NPC] = x[lo:lo + NPC]
        m["xr"] = xr
        in_maps.append(m)
    return in_maps


def _build_nc(w1, b1, w2, b2):
    import concourse.bacc as bacc
    import concourse.bass as bass
    import concourse.mybir as mybir
    import concourse.tile as tile

    f32 = mybir.dt.float32
    i16 = mybir.dt.int16

    nc = bacc.Bacc(None, target_bir_lowering=False,
                   dynamic_dma_scratch_size=DT_SCRATCH)

    xpad = nc.dram_tensor("xpad", [XROWS, DPAD], f32, kind="ExternalInput")
    xr = nc.dram_tensor("xr", [EPAD, D], f32, kind="ExternalInput")
    gi, wv, ei, pa, mk = {}, {}, {}, {}, {}
    csum = {}
    for d in range(2):
        for c in range(NCHUNK):
            s = f"{d}{c}"
            gi[s] = nc.dram_tensor(f"gi_{s}", [128, LSTREAM // 16], i16, kind="ExternalInput")
            wv[s] = nc.dram_tensor(f"wv_{s}", [128, LSTREAM // 128], f32, kind="ExternalInput")
            ei[s] = nc.dram_tensor(f"ei_{s}", [128, EPAD // 16], i16, kind="ExternalInput")
            pa[s] = nc.dram_tensor(f"pa_{s}", [128, EP_T], f32, kind="ExternalInput")
            mk[s] = nc.dram_tensor(f"mk_{s}", [128, EP_T], f32, kind="ExternalInput")
            csum[s] = nc.dram_tensor(f"csum_{s}", [LSTREAM // 2, DPAD], f32)
    out = nc.dram_tensor("out", [EPAD, O], f32, kind="ExternalOutput")

    # constants baked as dram inputs would be cleaner, but weights are tiny:
    w1_t = nc.dram_tensor("w1", [3 * D, O], f32, kind="ExternalInput")
    b1_t = nc.dram_tensor("b1", [O, 1], f32, kind="ExternalInput")
    w2_t = nc.dram_tensor("w2", [O, O], f32, kind="ExternalInput")
    b2_t = nc.dram_tensor("b2", [O, 1], f32, kind="ExternalInput")
    tri_t = nc.dram_tensor("tri", [128, 128], f32, kind="ExternalInput")
    id_t = nc.dram_tensor("ident", [128, 128], f32, kind="ExternalInput")

    with tile.TileContext(nc) as tc:
        with (
            tc.tile_pool(name="const", bufs=1) as constp,
            tc.tile_pool(name="gidx", bufs=3) as gip,
            tc.tile_pool(name="gath", bufs=2) as gp,
            tc.tile_pool(name="gw", bufs=4) as gwp,
            tc.tile_pool(name="evac", bufs=4) as evp,
            tc.tile_pool(name="bnd", bufs=1) as bp,
            tc.tile_pool(name="acc", bufs=1) as accp,
            tc.tile_pool(name="mlp", bufs=3) as mlpp,
            tc.tile_pool(name="ps", bufs=2, space="PSUM") as psp,
            tc.tile_pool(name="psb", bufs=1, space="PSUM") as psb,
            tc.tile_pool(name="ps2", bufs=1, space="PSUM") as psp2,
        ):
            trit = constp.tile([128, 128], f32)
            idt = constp.tile([128, 128], f32)
            w1t = constp.tile([3 * D, O], f32)
            b1t = constp.tile([O, 1], f32)
            w2t = constp.tile([O, O], f32)
            b2t = constp.tile([O, 1], f32)
            nc.sync.dma_start(out=trit[:], in_=tri_t[:])
            nc.sync.dma_start(out=idt[:], in_=id_t[:])
            nc.sync.dma_start(out=w1t[:], in_=w1_t[:])
            nc.sync.dma_start(out=b1t[:], in_=b1_t[:])
            nc.sync.dma_start(out=w2t[:], in_=w2_t[:])
            nc.sync.dma_start(out=b2t[:], in_=b2_t[:])

            acc0 = accp.tile([128, EP_T, D], f32, tag="acc0")
            acc1 = accp.tile([128, EP_T, D], f32, tag="acc1")
            acc = {0: acc0, 1: acc1}

            for d in range(2):
                for c in range(NCHUNK):
                    s = f"{d}{c}"
                    wvt = gip.tile([128, LSTREAM // 128], f32, tag="wv")
                    nc.sync.dma_start(out=wvt[:], in_=wv[s][:])
                    csv = csum[s].rearrange(
                        "(s p th) (tl e) -> s p (th tl) e", p=128, th=8, tl=2)
                    for g in range(LSTREAM // GK):
                        git = gip.tile([128, GK // 16], i16, tag="gi")
                        nc.sync.dma_start(
                            out=git[:],
                            in_=gi[s][:, g * (GK // 16):(g + 1) * (GK // 16)])
                        gt = gp.tile([128, GK // 128, DPAD], f32, tag="g")
                        nc.gpsimd.dma_gather(
                            out_ap=gt[:],
                            in_ap=xpad[c * CH:(c + 1) * CH, :],
                            idxs_ap=git[:],
                            num_idxs=GK, num_idxs_reg=GK, elem_size=DPAD,
                            single_packet=False)
                        for si in range(GK // SUP):
                            sg = g * (GK // SUP) + si
                            gwt = gwp.tile([128, SUP // 128, D], f32, tag="gw")
                            wsl = wvt[:, sg * (SUP // 128):(sg + 1) * (SUP // 128)]
                            wb = bass.AP(
                                wsl.tensor, wsl.offset,
                                [wsl.ap[0], [1, SUP // 128], [0, D]])
                            nc.vector.tensor_tensor(
                                out=gwt[:],
                                in0=gt[:, si * (SUP // 128):(si + 1) * (SUP // 128), :D],
                                in1=wb, op=mybir.AluOpType.mult)
                            pst = psp.tile([128, 512], f32, space="PSUM", tag="pfx")
                            nc.tensor.matmul(
                                pst[:], lhsT=trit[:],
                                rhs=gwt[:].rearrange("p t e -> p (t e)"),
                                start=True, stop=True)
                            ev = evp.tile([128, SUP // 128, D], f32, tag="ev")
                            nc.vector.tensor_copy(
                                out=ev[:].rearrange("p t e -> p (t e)"), in_=pst[:])
                            nc.sync.dma_start(out=csv[sg], in_=ev[:])

                    # ---- boundary: gather prefix rows at segment ends
                    ept = bp.tile([128, EP_T, DPAD], f32, tag="ep")
                    for h in range(BG_SPLIT):
                        eit = gip.tile([128, BG_K // 16], i16, tag="ei")
                        nc.sync.dma_start(
                            out=eit[:],
                            in_=ei[s][:, h * (BG_K // 16):(h + 1) * (BG_K // 16)])
                        nc.gpsimd.dma_gather(
                            out_ap=ept[:, h * (EP_T // BG_SPLIT):(h + 1) * (EP_T // BG_SPLIT), :],
                            in_ap=csum[s][:],
                            idxs_ap=eit[:],
                            num_idxs=BG_K, num_idxs_reg=BG_K, elem_size=DPAD,
                            single_packet=False)
                    pat = bp.tile([128, EP_T], f32, tag="pa")
                    mkt = bp.tile([128, EP_T], f32, tag="mk")
                    nc.sync.dma_start(out=pat[:], in_=pa[s][:])
                    nc.sync.dma_start(out=mkt[:], in_=mk[s][:])
                    # E = A + par*(B - A)
                    et = bp.tile([128, EP_T, D], f32, tag="E")
                    nc.vector.tensor_tensor(
                        out=et[:], in0=ept[:, :, D:2 * D], in1=ept[:, :, 0:D],
                        op=mybir.AluOpType.subtract)
                    pab = bass.AP(pat.tensor, pat[:].offset,
                                  [pat[:].ap[0], [1, EP_T], [0, D]])
                    nc.vector.tensor_tensor(
                        out=et[:], in0=et[:], in1=pab, op=mybir.AluOpType.mult)
                    nc.vector.tensor_tensor(
                        out=et[:], in0=et[:], in1=ept[:, :, 0:D],
                        op=mybir.AluOpType.add)
                    # Eprev = E shifted by one slot (partition shift via DMA)
                    evp_t = bp.tile([128, EP_T, D], f32, tag="Ep")
                    nc.sync.dma_start(out=evp_t[1:128, :, :], in_=et[0:127, :, :])
                    nc.sync.dma_start(out=evp_t[0:1, 1:EP_T, :], in_=et[127:128, 0:EP_T - 1, :])
                    nc.vector.memset(evp_t[0:1, 0:1, :], 0.0)
                    # acc += E - mask * Eprev
                    mkb = bass.AP(mkt.tensor, mkt[:].offset,
                                  [mkt[:].ap[0], [1, EP_T], [0, D]])
                    nc.vector.tensor_tensor(
                        out=evp_t[:], in0=evp_t[:], in1=mkb,
                        op=mybir.AluOpType.mult)
                    nc.vector.tensor_tensor(
                        out=et[:], in0=et[:], in1=evp_t[:],
                        op=mybir.AluOpType.subtract)
                    if c == 0:
                        nc.vector.tensor_copy(out=acc[d][:], in_=et[:])
                    else:
                        nc.vector.tensor_tensor(
                            out=acc[d][:], in0=acc[d][:], in1=et[:],
                            op=mybir.AluOpType.add)

            # ---------------------------------------------------- MLP phase
            xrt = accp.tile([128, EP_T, D], f32, tag="xr")
            nc.sync.dma_start(
                out=xrt[:],
                in_=xr.rearrange("(t p) e -> p t e", p=128))
            NC_CH = 256               # nodes per MLP chunk
            for j in range(EPAD // NC_CH):
                mtc = mlpp.tile([3 * D, NC_CH], f32, tag="mt")
                for q in range(NC_CH // 128):
                    nch = j * (NC_CH // 128) + q
                    for fi, tsrc in enumerate((acc[0], acc[1], xrt)):
                        pt = psp2.tile([D, 128], f32, space="PSUM", tag="tp")
                        nc.tensor.transpose(
                            out=pt[:], in_=tsrc[:, nch, :], identity=idt[:])
                        nc.vector.tensor_copy(
                            out=mtc[fi * D:(fi + 1) * D, q * 128:(q + 1) * 128],
                            in_=pt[:])
                h1p = psb.tile([O, NC_CH], f32, space="PSUM", tag="h1")
                nc.tensor.matmul(h1p[:], lhsT=w1t[:], rhs=mtc[:],
                                 start=True, stop=True)
                h1t = mlpp.tile([O, NC_CH], f32, tag="h1s")
                nc.scalar.activation(out=h1t[:], in_=h1p[:],
                                     func=mybir.ActivationFunctionType.Tanh,
                                     bias=b1t[:], scale=1.0)
                o2p = psb.tile([O, NC_CH], f32, space="PSUM", tag="o2")
                nc.tensor.matmul(o2p[:], lhsT=w2t[:], rhs=h1t[:],
                                 start=True, stop=True)
                o2t = mlpp.tile([O, NC_CH], f32, tag="o2s")
                nc.scalar.activation(out=o2t[:], in_=o2p[:],
                                     func=mybir.ActivationFunctionType.Tanh,
                                     bias=b2t[:], scale=1.0)
                # transpose back to node-major and store
                ost = mlpp.tile([128, NC_CH // 128, O], f32, tag="os")
                for q in range(NC_CH // 128):
                    po = psp2.tile([128, O], f32, space="PSUM", tag="tp2")
                    nc.tensor.transpose(
                        out=po[:], in_=o2t[:, q * 128:(q + 1) * 128],
                        identity=idt[0:O, 0:O])
                    nc.vector.tensor_copy(out=ost[:, q, :], in_=po[:])
                nc.sync.dma_start(
                    out=out.rearrange("(t p) e -> p t e", p=128)[
                        :, j * (NC_CH // 128):(j + 1) * (NC_CH // 128), :],
                    in_=ost[:])
    nc.compile()
    return nc


_CACHE = {}


def kernel(x, edge_index, edge_attr, W1, b1, W2, b2):
    from concourse.bass_utils import run_bass_kernel_spmd

    x = np.asarray(x, np.float32)
    in_maps = _host_prep(x, edge_index, edge_attr)

    tri = np.triu(np.ones((128, 128), np.float32))  # lhsT[k,p]=1 for k<=p
    ident = np.eye(128, dtype=np.float32)
    shared = {
        "w1": np.asarray(W1, np.float32),
        "b1": np.asarray(b1, np.float32).reshape(O, 1),
        "w2": np.asarray(W2, np.float32),
        "b2": np.asarray(b2, np.float32).reshape(O, 1),
        "tri": tri, "ident": ident,
    }
    for m in in_maps:
        m.update(shared)

    if "nc" not in _CACHE:
        _CACHE["nc"] = _build_nc(W1, b1, W2, b2)
    nc = _CACHE["nc"]
    res = run_bass_kernel_spmd(nc, in_maps, core_ids=list(range(NCORES)))
    out = np.concatenate([res.results[k]["out"][:NPC] for k in range(NCORES)], axis=0)
    return out.astype(np.float32)
